# revision 60
# baseline (speedup 1.0000x reference)
"""GAT (3-layer, 4-head) graph-classification kernel for 8 Trainium2 NeuronCores.

Strategy (dst-sharded message passing, gather-packet-floor oriented):
  - Nodes are degree-sorted and dealt round-robin to 8 cores (graph/data
    parallel). Within a core, nodes are split into two table halves (H1/H2,
    each < 32768 rows so int16 gather indices work), tiled into 128-node
    destination tiles packed to minimize per-tile edge-slot padding
    (d1-sorted chunks with within-chunk d2 concentration).
  - Per layer: each core computes h|al_src|al_dst for its shard with one
    matmul (x_T @ [W | W@Asrc | W@Adst]), stages packed 768 B bf16 rows to a
    local HBM shard, and the halves are replicated with TWO AllGathers (into
    Local DRAM — Shared DRAM reads are ~4x slower) so the H2 collective
    overlaps the H1 edge gathers.
  - Edge phase: per-edge source rows are fetched with GPSIMD dma_gather
    (the dominant cost: ~9-11 ns per 768 B row-packet, spread over 4 SWDGE
    queues; issue is software-pipelined so the in-order Pool stream never
    stalls on the H2 allgather). Attention softmax runs per destination on
    VectorE/ScalarE with no max-shift (logits are provably tiny for these
    inputs); self-loops are never gathered — their exact f32 contribution is
    computed from locally resident phase-A outputs. Messages are
    alpha-weighted in place and segment-summed (contiguous pre-halving +
    short strided reduce).
  - Layer outputs are transposed back to feature-major (TensorE) to feed the
    next layer's matmul; after layer 3 a one-hot matmul pools node features
    into per-graph sums. Host sums the 8 per-core partial graph outputs.
"""

import sys

for _p in ("/opt/trn_rl_repo",):
    if _p not in sys.path:
        sys.path.insert(0, _p)

import numpy as np
import ml_dtypes

import concourse.bass as bass
import concourse.bacc as bacc
import concourse.mybir as mybir
import concourse.tile as tile
from concourse import library_config
from concourse.bass_utils import run_bass_kernel_spmd

FP = mybir.dt.float32
BF = mybir.dt.bfloat16
I16 = mybir.dt.int16
BFNP = ml_dtypes.bfloat16

# Problem constants (hardcoded per the harness contract).
N = 50000
E = 800000
IN = 128
H = 4
D = 64
HD = 256
G = 64
NEG = 0.2

NCORES = 8
TILES = 49                 # 128-node tiles per core
SHARD = TILES * 128        # 6272 rows per core (6250 real + 22 pad)
ROWW = 384                 # bf16 columns per table row (768 B): h[256] | al_src f32[4] | pad
                           # (dma_gather requires elem_size % 256 B == 0)
TILES_H1 = 25              # tiles per core in table half 1 (allgathered first)
TILES_H2 = TILES - TILES_H1
H1ROWS = TILES_H1 * 128    # 3200 rows/core; full H1 table = 25600 (< int16 max)
H2ROWS = TILES_H2 * 128    # 3072 rows/core; full H2 table = 24576
CMAX = 38                  # max slot columns per tile-group
TMAX = 1                   # max 128-dst tiles per group (T=1: exact per-tile
                           # slot widths; the widest single tile already sets
                           # the hx buffer size, so grouping saves no SBUF)
QB = 4                     # phase-A chunks per staging DMA
NEGINF = -1.0e30

_cache = {}


# ----------------------------------------------------------------------------
# Host-side preprocessing
# ----------------------------------------------------------------------------

def _preprocess(edge_index, batch):
    # Self-loops are NOT materialized as edges: each node's own contribution
    # is computed on-device from locally resident phase-A outputs.
    src = np.asarray(edge_index[0], np.int64)
    dst = np.asarray(edge_index[1], np.int64)
    deg = np.bincount(dst, minlength=N)  # in-degree excluding self-loop

    # pass 1: deal nodes to cores by total-degree rank (load balance). Within
    # each core the top H1ROWS nodes by degree form table-half H1 (tiles
    # 0..TILES_H1), the rest H2 — H1 is allgathered first so edge gathers can
    # start while H2 is still in flight.
    order1 = np.argsort(-deg, kind="stable")
    core_of = np.empty(N, np.int64)
    core_of[order1] = np.arange(N) % NCORES
    ish1 = np.zeros(N, bool)
    for c in range(NCORES):
        nodes = order1[c::NCORES]
        ish1[nodes[:H1ROWS]] = True
    srcH1 = ish1[src]
    d1 = np.bincount(dst[srcH1], minlength=N)
    d2 = deg - d1

    # pass 2: within each core sort each half by (d1, d2) desc so that each
    # 128-node tile is near-uniform in BOTH per-region degrees.
    core_nodes = np.full((NCORES, TILES * 128), -1, np.int64)
    rowloc = np.full(N, -1, np.int64)  # t*128 + p within the core
    def pack_half(nh):
        # d1-sorted chunks of 4 tiles; within each chunk concentrate high-d2
        # nodes into as few tiles as possible (d2-desc) so fewer tiles pay a
        # wide region-2 slot budget.
        nh = nh[np.lexsort((-d2[nh], -d1[nh]))]
        out = []
        for s in range(0, len(nh), 512):
            chunk = nh[s : s + 512]
            out.append(chunk[np.argsort(-d2[chunk], kind="stable")])
        return np.concatenate(out) if out else nh

    for c in range(NCORES):
        nodes = order1[c::NCORES]
        n1 = pack_half(nodes[:H1ROWS])
        n2 = pack_half(nodes[H1ROWS:])
        core_nodes[c, :H1ROWS] = n1
        core_nodes[c, H1ROWS : H1ROWS + len(n2)] = n2
        rowloc[n1] = np.arange(H1ROWS)
        rowloc[n2] = H1ROWS + np.arange(len(n2))

    # per-edge region-local table row of the source
    c_s = core_of[src]
    rl = rowloc[src]
    arow_e = np.where(ish1[src], c_s * H1ROWS + rl, c_s * H2ROWS + rl - H1ROWS)
    region = (~srcH1).astype(np.int64)
    eorder = np.lexsort((region, dst))
    arow_by = arow_e[eorder]
    reg_by = region[eorder]
    dst_by = dst[eorder]
    starts = np.searchsorted(dst_by, np.arange(N))

    # per-tile slot widths, shared across cores for SPMD
    LA = np.zeros(TILES, np.int64)
    LB = np.zeros(TILES, np.int64)
    for t in range(TILES):
        nodes_t = core_nodes[:, t * 128 : (t + 1) * 128].reshape(-1)
        real = nodes_t >= 0
        if real.any():
            LA[t] = d1[nodes_t[real]].max()
            LB[t] = d2[nodes_t[real]].max()

    groups = []
    t = 0
    while t < TILES:
        T = 1
        while T < TMAX and t + T < TILES:
            nLA = LA[t : t + T + 1].max()
            nLB = LB[t : t + T + 1].max()
            if (T + 1) * (nLA + nLB) <= CMAX:
                T += 1
            else:
                break
        groups.append((t, T, int(LA[t : t + T].max()), int(LB[t : t + T].max())))
        t += T

    tot_slots = sum(T * 128 * (gLA + gLB) for (_, T, gLA, gLB) in groups)
    n_edges = E

    # per-core packed idx / mask arrays
    XI = sum((T * gLA + T * gLB) * 8 for (_, T, gLA, gLB) in groups)
    XM = sum(T * (gLA + gLB) for (_, T, gLA, gLB) in groups)
    idx_all = np.zeros((NCORES, 128, XI), np.int16)
    mask_all = np.full((NCORES, 128, XM), NEGINF, np.float32)
    goffs = []  # (idx colA off, idx colB off, mask col off) per group

    for c in range(NCORES):
        io = 0
        mo = 0
        for gi, (t0, T, gLA, gLB) in enumerate(groups):
            if c == 0:
                goffs.append((io, io + T * gLA * 8, mo))
            CA, CB = T * gLA, T * gLB
            blkA = np.zeros((T * gLA, 128), np.int16)
            blkB = np.zeros((T * gLB, 128), np.int16)
            for ti in range(T):
                nodes_t = core_nodes[c, (t0 + ti) * 128 : (t0 + ti + 1) * 128]
                safe = np.maximum(nodes_t, 0)
                dA = np.where(nodes_t >= 0, d1[safe], 0)
                dB = np.where(nodes_t >= 0, d2[safe], 0)
                st = starts[safe]
                if gLA:
                    ji = st[:, None] + np.arange(gLA)[None, :]
                    vals = arow_by[np.minimum(ji, n_edges - 1)]
                    valid = np.arange(gLA)[None, :] < dA[:, None]
                    vals = np.where(valid, vals, 0)
                    blkA[ti * gLA : (ti + 1) * gLA, :] = vals.T.astype(np.int16)
                    mask_all[c, :, mo + ti * gLA : mo + (ti + 1) * gLA] = np.where(
                        valid, 0.0, NEGINF
                    )
                if gLB:
                    ji = st[:, None] + dA[:, None] + np.arange(gLB)[None, :]
                    vals = arow_by[np.minimum(ji, n_edges - 1)]
                    valid = np.arange(gLB)[None, :] < dB[:, None]
                    vals = np.where(valid, vals, 0)
                    blkB[ti * gLB : (ti + 1) * gLB, :] = vals.T.astype(np.int16)
                    mask_all[
                        c, :, mo + CA + ti * gLB : mo + CA + (ti + 1) * gLB
                    ] = np.where(valid, 0.0, NEGINF)
            if gLA:
                w = blkA.reshape(-1).reshape(-1, 16).T  # [16, CA*8]
                idx_all[c, :, io : io + CA * 8] = np.tile(w, (8, 1))
                io += CA * 8
            if gLB:
                w = blkB.reshape(-1).reshape(-1, 16).T
                idx_all[c, :, io : io + CB * 8] = np.tile(w, (8, 1))
                io += CB * 8
            mo += CA + CB
        assert io == XI and mo == XM

    # pooling one-hot [p, t*G + g]
    onehot = np.zeros((NCORES, 128, TILES * G), np.float32)
    for c in range(NCORES):
        nodes = core_nodes[c]
        real = nodes >= 0
        tt = np.arange(TILES * 128) // 128
        pp = np.arange(TILES * 128) % 128
        gid = batch[np.maximum(nodes, 0)]
        onehot[c, pp[real], tt[real] * G + gid[real]] = 1.0

    return dict(
        core_nodes=core_nodes,
        groups=groups,
        goffs=goffs,
        idx_all=idx_all,
        mask_all=mask_all,
        onehot=onehot,
        XI=XI,
        XM=XM,
        tot_slots=tot_slots,
    )


def _build_wcat(W, a_src, a_dst):
    F = W.shape[0]
    Asrc = np.zeros((HD, H), np.float64)
    Adst = np.zeros((HD, H), np.float64)
    for h in range(H):
        Asrc[h * D : (h + 1) * D, h] = a_src[h]
        Adst[h * D : (h + 1) * D, h] = a_dst[h]
    Wc = np.zeros((F, 264), np.float64)
    Wc[:, 0:256] = W
    Wc[:, 256:260] = W @ Asrc
    Wc[:, 260:264] = W @ Adst
    return Wc.astype(BFNP)


# ----------------------------------------------------------------------------
# Bass program
# ----------------------------------------------------------------------------

def _build_program(meta, stage=3, repeat=1, estage=None):
    groups = meta["groups"]
    goffs = meta["goffs"]
    XI, XM = meta["XI"], meta["XM"]
    CMAXG = max(T * (gLA + gLB) for (_, T, gLA, gLB) in groups)
    TMAXG = max(T for (_, T, _, _) in groups)

    nc = bacc.Bacc(
        "TRN2",
        target_bir_lowering=False,
        debug=False,
        enable_asserts=False,
        num_devices=NCORES,
        num_swdge_queues=4,
    )

    d_x0T = nc.dram_tensor("x0T", [IN, SHARD], BF, kind="ExternalInput")
    d_wcat = [
        nc.dram_tensor(f"wcat{l}", [128 if l == 0 else 256, 264], BF, kind="ExternalInput")
        for l in range(3)
    ]
    d_bias = [
        nc.dram_tensor(f"bias{l}", [128, 256], FP, kind="ExternalInput") for l in range(3)
    ]
    d_ident = nc.dram_tensor("ident", [128, 128], BF, kind="ExternalInput")
    d_idx = nc.dram_tensor("idxall", [128, XI], I16, kind="ExternalInput")
    d_mask = nc.dram_tensor("maskall", [128, XM], FP, kind="ExternalInput")
    d_onehot = nc.dram_tensor("onehot", [128, TILES * G], BF, kind="ExternalInput")
    d_out = nc.dram_tensor("pooled", [G, HD], FP, kind="ExternalOutput")

    with tile.TileContext(nc) as tc:
        nc.gpsimd.load_library(library_config.mlp)
        with (
            tc.tile_pool(name="const", bufs=1) as cpool,
            tc.tile_pool(name="gath", bufs=3) as gpool,
            tc.tile_pool(name="att", bufs=3) as epool,
            tc.tile_pool(name="stage", bufs=2) as spool,
            tc.tile_pool(name="og", bufs=2) as ogpool,
            tc.tile_pool(name="psA", bufs=2, space="PSUM") as pspool,
            tc.tile_pool(name="psT", bufs=2, space="PSUM") as pstp,
            tc.tile_pool(name="psP", bufs=1, space="PSUM") as ppool,
            tc.tile_pool(name="dram", bufs=1, space="DRAM") as dpool,
        ):
            # resident tiles
            xT_a = cpool.tile([128, SHARD], BF, tag="xTa")
            xT_b = cpool.tile([128, SHARD], BF, tag="xTb")
            wcat_sb = []
            for l in range(3):
                ks = 1 if l == 0 else 2
                tiles_l = [
                    cpool.tile([128, 264], BF, name=f"wc{l}{k}", tag=f"wc{l}{k}")
                    for k in range(ks)
                ]
                wcat_sb.append(tiles_l)
            bias_sb = [cpool.tile([128, 256], FP, name=f"b{l}", tag=f"b{l}") for l in range(3)]
            ident = cpool.tile([128, 128], BF, tag="ident")
            idx_sb = cpool.tile([128, XI], I16, tag="idx")
            mask_sb = cpool.tile([128, XM], FP, tag="mask")
            onehot_sb = cpool.tile([128, TILES * G], BF, tag="oneh")
            aldst = cpool.tile([128, TILES * 4], FP, tag="aldst")
            alsrc_own = cpool.tile([128, TILES * 4], FP, tag="alsrco")
            esf_all = cpool.tile([128, TILES * 4], FP, tag="esfall")
            esf_lr = cpool.tile([128, TILES * 4], FP, tag="esflr")
            h_own = cpool.tile([128, TILES * 256], BF, tag="hown")

            # tablefull is Local, NOT Shared: local DMA gathers read Shared
            # DRAM at ~1/4 bandwidth, which dominates everything else. The
            # AllGather's non-Shared-output path is slower per-collective but
            # the gather speedup wins by far. The table is allgathered in two
            # halves (H1 = tiles 0..24, H2 = rest) so H2's collective overlaps
            # the H1 edge gathers.
            tableshards = [
                dpool.tile(
                    [SHARD, ROWW], BF, name=f"tshard{lr}", tag=f"tshard{lr}"
                )
                for lr in range(3 * repeat)
            ]
            tablefullH1 = [
                dpool.tile(
                    [NCORES * H1ROWS, ROWW],
                    BF,
                    name=f"tfh1_{lr}",
                    tag=f"tfh1_{lr}",
                )
                for lr in range(3 * repeat)
            ]
            tablefullH2 = [
                dpool.tile(
                    [NCORES * H2ROWS, ROWW],
                    BF,
                    name=f"tfh2_{lr}",
                    tag=f"tfh2_{lr}",
                )
                for lr in range(3 * repeat)
            ]

            # constant loads
            nc.sync.dma_start(xT_a[:], d_x0T[:])
            for l in range(3):
                for k, wt in enumerate(wcat_sb[l]):
                    nc.sync.dma_start(wt[:], d_wcat[l][k * 128 : (k + 1) * 128, :])
                nc.sync.dma_start(bias_sb[l][:], d_bias[l][:])
            nc.sync.dma_start(ident[:], d_ident[:])
            nc.sync.dma_start(idx_sb[:], d_idx[:])
            nc.sync.dma_start(mask_sb[:], d_mask[:])
            nc.sync.dma_start(onehot_sb[:], d_onehot[:])

            # estage: None = full kernel. Profiling cutoffs (always 3 layers):
            #   0 = phase A only (no allgather), 9 = +allgather,
            #   10 = +edge gathers, 11 = +attention softmax,
            #   12 = +messages/segment-sum/bias/relu, 13 = +writeback.
            full = estage is None
            if not full:
                nc.vector.memset(xT_b[:], 0)
            nlayers = 3
            reps = repeat
            for rep in range(reps):
              pool_ps = (
                ppool.tile([64, 256], FP, name="pool_ps", tag="poolps")
                if full
                else None
              )
              for l in range(nlayers):
                ks = 1 if l == 0 else 2
                tableshard = tableshards[rep * 3 + l]
                tabA = tablefullH1[rep * 3 + l]
                tabB = tablefullH2[rep * 3 + l]
                # shard rows are tile-major: row = q*128 + p
                tsh3 = tableshard.rearrange("(q p) w -> p q w", p=128)
                # ---------------- phase A: node transform + table shard ----
                for q0 in range(0, TILES, QB):
                    nq = min(QB, TILES - q0)
                    stg = spool.tile([128, QB * ROWW], BF, tag="stg")
                    stg3 = stg[:].rearrange("p (q w) -> p q w", w=ROWW)
                    stgf = stg[:].bitcast(FP).rearrange("p (q w) -> p q w", w=ROWW // 2)
                    nc.vector.memset(stg3[:, :, 264:ROWW], 0)
                    for qi in range(nq):
                        q = q0 + qi
                        ps = pspool.tile([128, 264], FP, tag="psA")
                        nc.tensor.matmul(
                            ps[:],
                            xT_a[:, q * 128 : (q + 1) * 128],
                            wcat_sb[l][0][:],
                            start=True,
                            stop=(ks == 1),
                        )
                        if ks == 2:
                            nc.tensor.matmul(
                                ps[:],
                                xT_b[:, q * 128 : (q + 1) * 128],
                                wcat_sb[l][1][:],
                                start=False,
                                stop=True,
                            )
                        nc.scalar.copy(stg3[:, qi, 0:256], ps[:, 0:256])
                        nc.scalar.copy(h_own[:, q * 256 : (q + 1) * 256], ps[:, 0:256])
                        nc.vector.tensor_copy(stgf[:, qi, 128:132], ps[:, 256:260])
                        nc.vector.tensor_copy(
                            alsrc_own[:, q * 4 : (q + 1) * 4], ps[:, 256:260]
                        )
                        nc.vector.tensor_copy(
                            aldst[:, q * 4 : (q + 1) * 4], ps[:, 260:264]
                        )
                    nc.sync.dma_start(
                        tsh3[:, q0 : q0 + nq, :], stg3[:, 0:nq, :]
                    )
                # self-loop weights, once per layer: exp(LRelu(alS_own+alD))
                nc.vector.tensor_add(esf_all[:], alsrc_own[:], aldst[:])
                nc.vector.tensor_scalar_min(esf_lr[:], esf_all[:], 0.0)
                nc.vector.tensor_scalar_max(esf_all[:], esf_all[:], 0.0)
                nc.vector.scalar_tensor_tensor(
                    esf_all[:],
                    esf_lr[:],
                    NEG,
                    esf_all[:],
                    op0=mybir.AluOpType.mult,
                    op1=mybir.AluOpType.add,
                )
                nc.scalar.activation(
                    esf_all[:], esf_all[:], mybir.ActivationFunctionType.Exp
                )
                if not full and estage < 9:
                    continue
                nc.gpsimd.collective_compute(
                    "AllGather",
                    mybir.AluOpType.bypass,
                    replica_groups=[list(range(NCORES))],
                    ins=[tableshard[0 : H1ROWS, :].opt()],
                    outs=[tabA.opt()],
                )
                # ---------------- edge phase -------------------------------
                if not full and estage < 10:
                    continue
                qctr = [0]

                # device limit: ≤1024 indices per dma_gather instruction
                def chunked_gather(hx, hx3, col0, ncols, tab, io):
                    # estage=14: timing probe — gather only the first
                    # 512B of each 768B row (h only, drops al_src)
                    probe = (not full) and estage == 14
                    ew = 256 if probe else ROWW
                    hv = (
                        hx[:, : (CMAXG * ROWW // 256) * 256].rearrange(
                            "p (c w) -> p c w", w=256
                        )
                        if probe
                        else hx3
                    )
                    tabv = tab[:, 0:256] if probe else tab
                    for k0 in range(0, ncols, 8):
                        kc = min(8, ncols - k0)
                        nc.gpsimd.dma_gather(
                            hv[:, col0 + k0 : col0 + k0 + kc, :],
                            tabv,
                            idx_sb[:, io + k0 * 8 : io + (k0 + kc) * 8],
                            kc * 128,
                            kc * 128,
                            ew,
                            elem_step=ROWW if probe else None,
                            queue_num=qctr[0] % 4,
                        )
                        qctr[0] += 1

                gstate = {}

                def issue_h1(gi):
                    t0, T, gLA, gLB = groups[gi]
                    ioA, ioB, mo = goffs[gi]
                    hx = gpool.tile([128, CMAXG * ROWW], BF, tag="hx")
                    hx3 = hx[:].rearrange("p (c w) -> p c w", w=ROWW)
                    gstate[gi] = (hx, hx3)
                    if gLA:
                        chunked_gather(hx, hx3, 0, T * gLA, tabA, ioA)

                def issue_h2(gi):
                    t0, T, gLA, gLB = groups[gi]
                    ioA, ioB, mo = goffs[gi]
                    hx, hx3 = gstate[gi]
                    if gLB:
                        chunked_gather(hx, hx3, T * gLA, T * gLB, tabB, ioB)

                def vector_ops(gi):
                    t0, T, gLA, gLB = groups[gi]
                    ioA, ioB, mo = goffs[gi]
                    CA, CB = T * gLA, T * gLB
                    C = CA + CB
                    hx, hx3 = gstate.pop(gi)

                    hxf = hx[:].bitcast(FP).rearrange("p (c w) -> p c w", w=ROWW // 2)
                    # alS[p, c, h] at f32 columns 128..132 of each row
                    e = epool.tile([128, CMAXG * 4], FP, tag="e")
                    if not full and estage in (10, 14):
                        nc.vector.tensor_copy(e[:, 0:64], hx3[:, 0, 0:64])
                        return
                    e3 = e[:].rearrange("p (c h) -> p c h", h=4)
                    ab = epool.tile([128, CMAXG * 4], BF, tag="ab")
                    ab3 = ab[:].rearrange("p (c h) -> p c h", h=4)

                    alD = aldst[:].rearrange("p (t h) -> p t h", h=4)[
                        :, t0 : t0 + T, :
                    ]

                    def reg_view(v3, off, L):
                        # [p, c, x] cols off..off+T*L -> [p, T, L, x]
                        return v3[:, off : off + T * L, :].rearrange(
                            "p (t j) h -> p t j h", j=L
                        )

                    regions = []
                    if gLA:
                        regions.append((0, gLA))
                    if gLB:
                        regions.append((CA, gLB))

                    # logits: e = al_src[src] + al_dst[dst]
                    for off, L in regions:
                        alS_r = hxf[:, off : off + T * L, 128:132].rearrange(
                            "p (t j) h -> p t j h", j=L
                        )
                        alD_b = alD.unsqueeze(2).broadcast_to((128, T, L, 4))
                        nc.vector.tensor_add(reg_view(e3, off, L), alS_r, alD_b)

                    eflat = e[:, : C * 4]
                    # leaky relu (composed: e = max(e,0) + NEG*min(e,0)), then pad mask
                    lr = epool.tile([128, CMAXG * 4], FP, name="lr", tag="lr")
                    lrf = lr[:, : C * 4]
                    nc.vector.tensor_scalar_min(lrf, eflat, 0.0)
                    nc.vector.tensor_scalar_max(eflat, eflat, 0.0)
                    nc.vector.scalar_tensor_tensor(
                        eflat,
                        lrf,
                        NEG,
                        eflat,
                        op0=mybir.AluOpType.mult,
                        op1=mybir.AluOpType.add,
                    )
                    mask_b = (
                        mask_sb[:, mo : mo + C].unsqueeze(2).broadcast_to((128, C, 4))
                    )
                    nc.vector.tensor_add(e3[:, 0:C, :], e3[:, 0:C, :], mask_b)

                    # no max-shift: logits are small (|e| < 8 for these
                    # inputs), so exp(e) is safe in f32 with huge margin and
                    # the whole segment-max pass is skipped
                    nc.scalar.activation(
                        eflat, eflat, mybir.ActivationFunctionType.Exp
                    )
                    esfv = esf_all[:, t0 * 4 : (t0 + T) * 4]

                    # denom and reciprocal
                    dt_ = []
                    for off, L in regions:
                        d_r = epool.tile([128, TMAXG * 4], FP, name=f"d{off == 0}", tag=f"d{off == 0}")
                        in_r = (
                            e3[:, off : off + T * L, :]
                            .rearrange("p (t j) h -> p t h j", j=L)
                        )
                        nc.vector.reduce_sum(
                            d_r[:, : T * 4], in_r, axis=mybir.AxisListType.X
                        )
                        dt_.append(d_r)
                    den = dt_[0]
                    if len(dt_) == 2:
                        nc.vector.tensor_add(
                            den[:, : T * 4], den[:, : T * 4], dt_[1][:, : T * 4]
                        )
                    nc.vector.tensor_add(den[:, : T * 4], den[:, : T * 4], esfv)
                    rec = epool.tile([128, TMAXG * 4], FP, tag="rec")
                    nc.vector.reciprocal(rec[:, : T * 4], den[:, : T * 4])
                    r3 = rec[:].rearrange("p (t h) -> p t h", h=4)[:, 0:T, :]
                    # alpha_self = ex_self / denom
                    nc.vector.tensor_mul(esfv, esfv, rec[:, : T * 4])

                    # alpha = ex / denom, cast to bf16
                    for off, L in regions:
                        r_b = r3.unsqueeze(2).broadcast_to((128, T, L, 4))
                        nc.vector.tensor_mul(
                            reg_view(e3, off, L), reg_view(e3, off, L), r_b
                        )
                    nc.vector.tensor_copy(ab[:, : C * 4], eflat)

                    if not full and estage == 11:
                        return
                    # messages: hx[:, :, 0:256] *= alpha (broadcast over 64)
                    h4 = hx3[:, 0:C, 0:256].rearrange("p c (h d) -> p c h d", d=D)
                    a4 = ab3[:, 0:C, :].unsqueeze(3).broadcast_to((128, C, 4, D))
                    nc.vector.tensor_mul(h4, h4, a4)
                    if not full and estage == 15:
                        return

                    # segment sum -> [p, T, 256]. Pre-halve with contiguous
                    # in-place adds (fast DVE mode) so the strided reduce only
                    # sees ~L/4 columns.
                    og = ogpool.tile([128, TMAXG * 256], FP, tag="ogA")
                    ogt = []
                    for off, L in regions:
                        o_r = (
                            og
                            if not ogt
                            else ogpool.tile(
                                [128, TMAXG * 256], FP, name="ogB", tag="ogB"
                            )
                        )
                        v = hx3[:, off : off + T * L, 0:256].rearrange(
                            "p (t j) f -> p t j f", j=L
                        )
                        L2 = L
                        for _ in range(2):
                            if L2 < 4:
                                break
                            h2 = L2 // 2
                            if L2 % 2:
                                nc.vector.tensor_add(
                                    v[:, :, 0, :], v[:, :, 0, :], v[:, :, L2 - 1, :]
                                )
                            nc.vector.tensor_add(
                                v[:, :, 0:h2, :],
                                v[:, :, 0:h2, :],
                                v[:, :, h2 : 2 * h2, :],
                            )
                            L2 = h2
                        in_r = v[:, :, 0:L2, :].rearrange("p t j f -> p t f j")
                        nc.vector.reduce_sum(
                            o_r[:, : T * 256], in_r, axis=mybir.AxisListType.X
                        )
                        ogt.append(o_r)
                    if len(ogt) == 2:
                        nc.vector.tensor_add(
                            og[:, : T * 256], og[:, : T * 256], ogt[1][:, : T * 256]
                        )
                    if not full and estage == 16:
                        return
                    # self message: og += h_own * alpha_self (relu_f doubles
                    # as the scratch; it is rewritten by the relu below)
                    relu_f = ogpool.tile([128, TMAXG * 256], FP, tag="reluf")
                    ho4 = h_own[:, t0 * 256 : (t0 + T) * 256].rearrange(
                        "p (t h d) -> p t h d", h=4, d=D
                    )
                    as4 = (
                        esf_all[:]
                        .rearrange("p (t h) -> p t h", h=4)[:, t0 : t0 + T, :]
                        .unsqueeze(3)
                        .broadcast_to((128, T, 4, D))
                    )
                    sm4 = relu_f[:].rearrange("p (t h d) -> p t h d", h=4, d=D)[
                        :, 0:T
                    ]
                    nc.vector.tensor_mul(sm4, ho4, as4)
                    nc.vector.tensor_add(
                        og[:, : T * 256], og[:, : T * 256], relu_f[:, : T * 256]
                    )

                    # bias + relu
                    og3 = og[:].rearrange("p (t f) -> p t f", f=256)
                    bias_b = bias_sb[l][:].unsqueeze(1).broadcast_to((128, T, 256))
                    nc.vector.tensor_add(og3[:, 0:T, :], og3[:, 0:T, :], bias_b)
                    nc.scalar.activation(
                        relu_f[:, : T * 256],
                        og[:, : T * 256],
                        mybir.ActivationFunctionType.Relu,
                    )

                    if not full and estage == 12:
                        return
                    relu_b = ogpool.tile([128, TMAXG * 256], BF, tag="relub")
                    nc.vector.tensor_copy(
                        relu_b[:, : T * 256], relu_f[:, : T * 256]
                    )
                    rb3 = relu_b[:].rearrange("p (t f) -> p t f", f=256)
                    if l < 2:
                        for ti in range(T):
                            for fb, xt in ((0, xT_a), (1, xT_b)):
                                pt = pstp.tile([128, 128], BF, tag="psT")
                                nc.tensor.transpose(
                                    pt[:],
                                    rb3[:, ti, fb * 128 : (fb + 1) * 128],
                                    ident[:],
                                )
                                nc.scalar.copy(
                                    xt[:, (t0 + ti) * 128 : (t0 + ti + 1) * 128],
                                    pt[:],
                                )
                    elif full:
                        for ti in range(T):
                            q = t0 + ti
                            nc.tensor.matmul(
                                pool_ps[:],
                                onehot_sb[:, q * G : (q + 1) * G],
                                rb3[:, ti, :],
                                start=(q == 0),
                                stop=(q == TILES - 1),
                            )

                # Software pipeline: keep H1 gathers KPRE groups ahead so the
                # Pool-engine stream never stalls at the H2-allgather wait
                # (an in-order stall there would idle every DMA queue).
                KPRE = 2
                NG = len(groups)
                for g in range(min(KPRE, NG)):
                    issue_h1(g)
                nc.gpsimd.collective_compute(
                    "AllGather",
                    mybir.AluOpType.bypass,
                    replica_groups=[list(range(NCORES))],
                    ins=[tableshard[H1ROWS:SHARD, :].opt()],
                    outs=[tabB.opt()],
                )
                for g in range(NG):
                    issue_h2(g)
                    if g + KPRE < NG:
                        issue_h1(g + KPRE)
                    vector_ops(g)

            pout = cpool.tile([64, 256], FP, tag="pout")
            if full:
                nc.vector.tensor_copy(pout[:], pool_ps[:])
            else:
                nc.vector.memset(pout[:], 0.0)
                nc.vector.tensor_add(pout[:, 0:196], pout[:, 0:196], aldst[0:64, 0:196])
            nc.sync.dma_start(d_out[:], pout[:])

    nc.compile()
    return nc


# ----------------------------------------------------------------------------
# Entry point
# ----------------------------------------------------------------------------

def _prepare(inputs):
    key = (
        inputs["edge_index"].tobytes(),
        inputs["batch"].tobytes(),
    )
    kh = hash(key)
    if kh in _cache:
        return _cache[kh]
    edge_index = np.asarray(inputs["edge_index"], np.int64)
    batch = np.asarray(inputs["batch"], np.int64)
    meta = _preprocess(edge_index, batch)
    nc = _build_program(meta)
    _cache[kh] = (meta, nc)
    return meta, nc


def _make_inmaps(inputs, meta):
    x = np.asarray(inputs["x"], np.float32)
    batch = np.asarray(inputs["batch"], np.int64)
    core_nodes = meta["core_nodes"]

    wcats = []
    biases = []
    for l in range(3):
        Wl = np.asarray(inputs[f"W{l}"], np.float64)
        wcats.append(
            _build_wcat(
                Wl,
                np.asarray(inputs[f"a_src{l}"], np.float64),
                np.asarray(inputs[f"a_dst{l}"], np.float64),
            )
        )
        b = np.asarray(inputs[f"b{l}"], np.float32)
        biases.append(np.tile(b[None, :], (128, 1)).astype(np.float32))
    ident = np.eye(128, dtype=BFNP)

    in_maps = []
    for c in range(NCORES):
        nodes = core_nodes[c]
        safe = np.maximum(nodes, 0)
        x0 = x[safe]
        x0[nodes < 0] = 0.0
        # column q*128+p = node (tile q, partition p); core_nodes is tile-major
        x0T = np.ascontiguousarray(x0.T).astype(BFNP)
        in_maps.append(
            {
                "x0T": x0T,
                "wcat0": wcats[0],
                "wcat1": wcats[1],
                "wcat2": wcats[2],
                "bias0": biases[0],
                "bias1": biases[1],
                "bias2": biases[2],
                "ident": ident,
                "idxall": meta["idx_all"][c],
                "maskall": meta["mask_all"][c],
                "onehot": meta["onehot"][c].astype(BFNP),
            }
        )
    return in_maps


def _run(inputs, trace=False):
    meta, nc = _prepare(inputs)
    in_maps = _make_inmaps(inputs, meta)
    res = run_bass_kernel_spmd(
        nc, in_maps, core_ids=list(range(NCORES)), trace=trace
    )
    out = np.zeros((G, HD), np.float64)
    for c in range(NCORES):
        out += res.results[c]["pooled"].astype(np.float64)
    return out.astype(np.float32), res


def kernel(**inputs) -> np.ndarray:
    out, _ = _run(inputs, trace=False)
    return out


def kernel_traced(**inputs):
    out, res = _run(inputs, trace=True)
    return out, res



# revision 61
# speedup vs baseline: 1.2329x; 1.2329x over previous
"""GAT (3-layer, 4-head) graph-classification kernel for 8 Trainium2 NeuronCores.

Strategy (dst-sharded message passing, gather-packet-floor oriented):
  - Nodes are degree-sorted and dealt round-robin to 8 cores (graph/data
    parallel). Within a core, nodes are split into two table halves (H1/H2,
    each < 32768 rows so int16 gather indices work), tiled into 128-node
    destination tiles packed to minimize per-tile edge-slot padding
    (d1-sorted chunks with within-chunk d2 concentration).
  - Per layer: each core computes h|al_src|al_dst for its shard with one
    matmul (x_T @ [W | W@Asrc | W@Adst]), stages packed 768 B bf16 rows to a
    local HBM shard, and the halves are replicated with TWO AllGathers (into
    Local DRAM — Shared DRAM reads are ~4x slower) so the H2 collective
    overlaps the H1 edge gathers.
  - Edge phase: per-edge source rows are fetched with GPSIMD dma_gather
    (the dominant cost: ~9-11 ns per 768 B row-packet, spread over 4 SWDGE
    queues; issue is software-pipelined so the in-order Pool stream never
    stalls on the H2 allgather). Attention softmax runs per destination on
    VectorE/ScalarE with no max-shift (logits are provably tiny for these
    inputs); self-loops are never gathered — their exact f32 contribution is
    computed from locally resident phase-A outputs. Messages are
    alpha-weighted in place and segment-summed (contiguous pre-halving +
    short strided reduce).
  - Layer outputs are transposed back to feature-major (TensorE) to feed the
    next layer's matmul; after layer 3 a one-hot matmul pools node features
    into per-graph sums. Host sums the 8 per-core partial graph outputs.
"""

import sys

for _p in ("/opt/trn_rl_repo",):
    if _p not in sys.path:
        sys.path.insert(0, _p)

import numpy as np
import ml_dtypes

import concourse.bass as bass
import concourse.bacc as bacc
import concourse.mybir as mybir
import concourse.tile as tile
from concourse import library_config
from concourse.bass_utils import run_bass_kernel_spmd

FP = mybir.dt.float32
BF = mybir.dt.bfloat16
I16 = mybir.dt.int16
BFNP = ml_dtypes.bfloat16

# Problem constants (hardcoded per the harness contract).
N = 50000
E = 800000
IN = 128
H = 4
D = 64
HD = 256
G = 64
NEG = 0.2

NCORES = 8
TILES = 49                 # 128-node tiles per core
SHARD = TILES * 128        # 6272 rows per core (6250 real + 22 pad)
ROWW = 384                 # bf16 columns per table row (768 B): h[256] | al_src f32[4] | pad
                           # (dma_gather requires elem_size % 256 B == 0)
TILES_H1 = 25              # tiles per core in table half 1 (allgathered first)
TILES_H2 = TILES - TILES_H1
H1ROWS = TILES_H1 * 128    # 3200 rows/core; full H1 table = 25600 (< int16 max)
H2ROWS = TILES_H2 * 128    # 3072 rows/core; full H2 table = 24576
CMAX = 38                  # max slot columns per tile-group
TMAX = 5                   # max 128-dst tiles per group (T=1 gives 2.8% fewer
                           # slots but measured slower — finer groups fragment
                           # the gather instruction stream)
QB = 4                     # phase-A chunks per staging DMA
NEGINF = -1.0e30

_cache = {}


# ----------------------------------------------------------------------------
# Host-side preprocessing
# ----------------------------------------------------------------------------

def _preprocess(edge_index, batch):
    # Self-loops are NOT materialized as edges: each node's own contribution
    # is computed on-device from locally resident phase-A outputs.
    src = np.asarray(edge_index[0], np.int64)
    dst = np.asarray(edge_index[1], np.int64)
    deg = np.bincount(dst, minlength=N)  # in-degree excluding self-loop

    # pass 1: deal nodes to cores by total-degree rank (load balance). Within
    # each core the top H1ROWS nodes by degree form table-half H1 (tiles
    # 0..TILES_H1), the rest H2 — H1 is allgathered first so edge gathers can
    # start while H2 is still in flight.
    order1 = np.argsort(-deg, kind="stable")
    core_of = np.empty(N, np.int64)
    core_of[order1] = np.arange(N) % NCORES
    ish1 = np.zeros(N, bool)
    for c in range(NCORES):
        nodes = order1[c::NCORES]
        ish1[nodes[:H1ROWS]] = True
    srcH1 = ish1[src]
    d1 = np.bincount(dst[srcH1], minlength=N)
    d2 = deg - d1

    # pass 2: within each core sort each half by (d1, d2) desc so that each
    # 128-node tile is near-uniform in BOTH per-region degrees.
    core_nodes = np.full((NCORES, TILES * 128), -1, np.int64)
    rowloc = np.full(N, -1, np.int64)  # t*128 + p within the core
    def pack_half(nh):
        # d1-sorted chunks of 4 tiles; within each chunk concentrate high-d2
        # nodes into as few tiles as possible (d2-desc) so fewer tiles pay a
        # wide region-2 slot budget.
        nh = nh[np.lexsort((-d2[nh], -d1[nh]))]
        out = []
        for s in range(0, len(nh), 512):
            chunk = nh[s : s + 512]
            out.append(chunk[np.argsort(-d2[chunk], kind="stable")])
        return np.concatenate(out) if out else nh

    for c in range(NCORES):
        nodes = order1[c::NCORES]
        n1 = pack_half(nodes[:H1ROWS])
        n2 = pack_half(nodes[H1ROWS:])
        core_nodes[c, :H1ROWS] = n1
        core_nodes[c, H1ROWS : H1ROWS + len(n2)] = n2
        rowloc[n1] = np.arange(H1ROWS)
        rowloc[n2] = H1ROWS + np.arange(len(n2))

    # per-edge region-local table row of the source
    c_s = core_of[src]
    rl = rowloc[src]
    arow_e = np.where(ish1[src], c_s * H1ROWS + rl, c_s * H2ROWS + rl - H1ROWS)
    region = (~srcH1).astype(np.int64)
    eorder = np.lexsort((region, dst))
    arow_by = arow_e[eorder]
    reg_by = region[eorder]
    dst_by = dst[eorder]
    starts = np.searchsorted(dst_by, np.arange(N))

    # per-tile slot widths, shared across cores for SPMD
    LA = np.zeros(TILES, np.int64)
    LB = np.zeros(TILES, np.int64)
    for t in range(TILES):
        nodes_t = core_nodes[:, t * 128 : (t + 1) * 128].reshape(-1)
        real = nodes_t >= 0
        if real.any():
            LA[t] = d1[nodes_t[real]].max()
            LB[t] = d2[nodes_t[real]].max()

    groups = []
    t = 0
    while t < TILES:
        T = 1
        while T < TMAX and t + T < TILES:
            nLA = LA[t : t + T + 1].max()
            nLB = LB[t : t + T + 1].max()
            if (T + 1) * (nLA + nLB) <= CMAX:
                T += 1
            else:
                break
        groups.append((t, T, int(LA[t : t + T].max()), int(LB[t : t + T].max())))
        t += T

    tot_slots = sum(T * 128 * (gLA + gLB) for (_, T, gLA, gLB) in groups)
    n_edges = E

    # per-core packed idx / mask arrays
    XI = sum((T * gLA + T * gLB) * 8 for (_, T, gLA, gLB) in groups)
    XM = sum(T * (gLA + gLB) for (_, T, gLA, gLB) in groups)
    idx_all = np.zeros((NCORES, 128, XI), np.int16)
    mask_all = np.full((NCORES, 128, XM), NEGINF, np.float32)
    goffs = []  # (idx colA off, idx colB off, mask col off) per group

    for c in range(NCORES):
        io = 0
        mo = 0
        for gi, (t0, T, gLA, gLB) in enumerate(groups):
            if c == 0:
                goffs.append((io, io + T * gLA * 8, mo))
            CA, CB = T * gLA, T * gLB
            blkA = np.zeros((T * gLA, 128), np.int16)
            blkB = np.zeros((T * gLB, 128), np.int16)
            for ti in range(T):
                nodes_t = core_nodes[c, (t0 + ti) * 128 : (t0 + ti + 1) * 128]
                safe = np.maximum(nodes_t, 0)
                dA = np.where(nodes_t >= 0, d1[safe], 0)
                dB = np.where(nodes_t >= 0, d2[safe], 0)
                st = starts[safe]
                if gLA:
                    ji = st[:, None] + np.arange(gLA)[None, :]
                    vals = arow_by[np.minimum(ji, n_edges - 1)]
                    valid = np.arange(gLA)[None, :] < dA[:, None]
                    vals = np.where(valid, vals, 0)
                    blkA[ti * gLA : (ti + 1) * gLA, :] = vals.T.astype(np.int16)
                    mask_all[c, :, mo + ti * gLA : mo + (ti + 1) * gLA] = np.where(
                        valid, 0.0, NEGINF
                    )
                if gLB:
                    ji = st[:, None] + dA[:, None] + np.arange(gLB)[None, :]
                    vals = arow_by[np.minimum(ji, n_edges - 1)]
                    valid = np.arange(gLB)[None, :] < dB[:, None]
                    vals = np.where(valid, vals, 0)
                    blkB[ti * gLB : (ti + 1) * gLB, :] = vals.T.astype(np.int16)
                    mask_all[
                        c, :, mo + CA + ti * gLB : mo + CA + (ti + 1) * gLB
                    ] = np.where(valid, 0.0, NEGINF)
            if gLA:
                w = blkA.reshape(-1).reshape(-1, 16).T  # [16, CA*8]
                idx_all[c, :, io : io + CA * 8] = np.tile(w, (8, 1))
                io += CA * 8
            if gLB:
                w = blkB.reshape(-1).reshape(-1, 16).T
                idx_all[c, :, io : io + CB * 8] = np.tile(w, (8, 1))
                io += CB * 8
            mo += CA + CB
        assert io == XI and mo == XM

    # pooling one-hot [p, t*G + g]
    onehot = np.zeros((NCORES, 128, TILES * G), np.float32)
    for c in range(NCORES):
        nodes = core_nodes[c]
        real = nodes >= 0
        tt = np.arange(TILES * 128) // 128
        pp = np.arange(TILES * 128) % 128
        gid = batch[np.maximum(nodes, 0)]
        onehot[c, pp[real], tt[real] * G + gid[real]] = 1.0

    return dict(
        core_nodes=core_nodes,
        groups=groups,
        goffs=goffs,
        idx_all=idx_all,
        mask_all=mask_all,
        onehot=onehot,
        XI=XI,
        XM=XM,
        tot_slots=tot_slots,
    )


def _build_wcat(W, a_src, a_dst):
    F = W.shape[0]
    Asrc = np.zeros((HD, H), np.float64)
    Adst = np.zeros((HD, H), np.float64)
    for h in range(H):
        Asrc[h * D : (h + 1) * D, h] = a_src[h]
        Adst[h * D : (h + 1) * D, h] = a_dst[h]
    Wc = np.zeros((F, 264), np.float64)
    Wc[:, 0:256] = W
    Wc[:, 256:260] = W @ Asrc
    Wc[:, 260:264] = W @ Adst
    return Wc.astype(BFNP)


# ----------------------------------------------------------------------------
# Bass program
# ----------------------------------------------------------------------------

def _build_program(meta, stage=3, repeat=1, estage=None):
    groups = meta["groups"]
    goffs = meta["goffs"]
    XI, XM = meta["XI"], meta["XM"]
    CMAXG = max(T * (gLA + gLB) for (_, T, gLA, gLB) in groups)
    TMAXG = max(T for (_, T, _, _) in groups)

    nc = bacc.Bacc(
        "TRN2",
        target_bir_lowering=False,
        debug=False,
        enable_asserts=False,
        num_devices=NCORES,
        num_swdge_queues=4,
    )

    d_x0T = nc.dram_tensor("x0T", [IN, SHARD], BF, kind="ExternalInput")
    d_wcat = [
        nc.dram_tensor(f"wcat{l}", [128 if l == 0 else 256, 264], BF, kind="ExternalInput")
        for l in range(3)
    ]
    d_bias = [
        nc.dram_tensor(f"bias{l}", [128, 256], FP, kind="ExternalInput") for l in range(3)
    ]
    d_ident = nc.dram_tensor("ident", [128, 128], BF, kind="ExternalInput")
    d_idx = nc.dram_tensor("idxall", [128, XI], I16, kind="ExternalInput")
    d_mask = nc.dram_tensor("maskall", [128, XM], FP, kind="ExternalInput")
    d_onehot = nc.dram_tensor("onehot", [128, TILES * G], BF, kind="ExternalInput")
    d_out = nc.dram_tensor("pooled", [G, HD], FP, kind="ExternalOutput")

    with tile.TileContext(nc) as tc:
        nc.gpsimd.load_library(library_config.mlp)
        with (
            tc.tile_pool(name="const", bufs=1) as cpool,
            tc.tile_pool(name="gath", bufs=3) as gpool,
            tc.tile_pool(name="att", bufs=3) as epool,
            tc.tile_pool(name="stage", bufs=2) as spool,
            tc.tile_pool(name="og", bufs=2) as ogpool,
            tc.tile_pool(name="psA", bufs=2, space="PSUM") as pspool,
            tc.tile_pool(name="psT", bufs=2, space="PSUM") as pstp,
            tc.tile_pool(name="psP", bufs=1, space="PSUM") as ppool,
            tc.tile_pool(name="dram", bufs=1, space="DRAM") as dpool,
        ):
            # resident tiles
            xT_a = cpool.tile([128, SHARD], BF, tag="xTa")
            xT_b = cpool.tile([128, SHARD], BF, tag="xTb")
            wcat_sb = []
            for l in range(3):
                ks = 1 if l == 0 else 2
                tiles_l = [
                    cpool.tile([128, 264], BF, name=f"wc{l}{k}", tag=f"wc{l}{k}")
                    for k in range(ks)
                ]
                wcat_sb.append(tiles_l)
            bias_sb = [cpool.tile([128, 256], FP, name=f"b{l}", tag=f"b{l}") for l in range(3)]
            ident = cpool.tile([128, 128], BF, tag="ident")
            idx_sb = cpool.tile([128, XI], I16, tag="idx")
            mask_sb = cpool.tile([128, XM], FP, tag="mask")
            onehot_sb = cpool.tile([128, TILES * G], BF, tag="oneh")
            aldst = cpool.tile([128, TILES * 4], FP, tag="aldst")
            alsrc_own = cpool.tile([128, TILES * 4], FP, tag="alsrco")
            esf_all = cpool.tile([128, TILES * 4], FP, tag="esfall")
            esf_lr = cpool.tile([128, TILES * 4], FP, tag="esflr")
            h_own = cpool.tile([128, TILES * 256], BF, tag="hown")

            # tablefull is Local, NOT Shared: local DMA gathers read Shared
            # DRAM at ~1/4 bandwidth, which dominates everything else. The
            # AllGather's non-Shared-output path is slower per-collective but
            # the gather speedup wins by far. The table is allgathered in two
            # halves (H1 = tiles 0..24, H2 = rest) so H2's collective overlaps
            # the H1 edge gathers.
            tableshards = [
                dpool.tile(
                    [SHARD, ROWW], BF, name=f"tshard{lr}", tag=f"tshard{lr}"
                )
                for lr in range(3 * repeat)
            ]
            tablefullH1 = [
                dpool.tile(
                    [NCORES * H1ROWS, ROWW],
                    BF,
                    name=f"tfh1_{lr}",
                    tag=f"tfh1_{lr}",
                )
                for lr in range(3 * repeat)
            ]
            tablefullH2 = [
                dpool.tile(
                    [NCORES * H2ROWS, ROWW],
                    BF,
                    name=f"tfh2_{lr}",
                    tag=f"tfh2_{lr}",
                )
                for lr in range(3 * repeat)
            ]

            # constant loads
            nc.sync.dma_start(xT_a[:], d_x0T[:])
            for l in range(3):
                for k, wt in enumerate(wcat_sb[l]):
                    nc.sync.dma_start(wt[:], d_wcat[l][k * 128 : (k + 1) * 128, :])
                nc.sync.dma_start(bias_sb[l][:], d_bias[l][:])
            nc.sync.dma_start(ident[:], d_ident[:])
            nc.sync.dma_start(idx_sb[:], d_idx[:])
            nc.sync.dma_start(mask_sb[:], d_mask[:])
            nc.sync.dma_start(onehot_sb[:], d_onehot[:])

            # estage: None = full kernel. Profiling cutoffs (always 3 layers):
            #   0 = phase A only (no allgather), 9 = +allgather,
            #   10 = +edge gathers, 11 = +attention softmax,
            #   12 = +messages/segment-sum/bias/relu, 13 = +writeback.
            full = estage is None
            if not full:
                nc.vector.memset(xT_b[:], 0)
            nlayers = 3
            reps = repeat
            for rep in range(reps):
              pool_ps = (
                ppool.tile([64, 256], FP, name="pool_ps", tag="poolps")
                if full
                else None
              )
              for l in range(nlayers):
                ks = 1 if l == 0 else 2
                tableshard = tableshards[rep * 3 + l]
                tabA = tablefullH1[rep * 3 + l]
                tabB = tablefullH2[rep * 3 + l]
                # shard rows are tile-major: row = q*128 + p
                tsh3 = tableshard.rearrange("(q p) w -> p q w", p=128)
                # ---------------- phase A: node transform + table shard ----
                for q0 in range(0, TILES, QB):
                    nq = min(QB, TILES - q0)
                    stg = spool.tile([128, QB * ROWW], BF, tag="stg")
                    stg3 = stg[:].rearrange("p (q w) -> p q w", w=ROWW)
                    stgf = stg[:].bitcast(FP).rearrange("p (q w) -> p q w", w=ROWW // 2)
                    nc.vector.memset(stg3[:, :, 264:ROWW], 0)
                    for qi in range(nq):
                        q = q0 + qi
                        ps = pspool.tile([128, 264], FP, tag="psA")
                        nc.tensor.matmul(
                            ps[:],
                            xT_a[:, q * 128 : (q + 1) * 128],
                            wcat_sb[l][0][:],
                            start=True,
                            stop=(ks == 1),
                        )
                        if ks == 2:
                            nc.tensor.matmul(
                                ps[:],
                                xT_b[:, q * 128 : (q + 1) * 128],
                                wcat_sb[l][1][:],
                                start=False,
                                stop=True,
                            )
                        nc.scalar.copy(stg3[:, qi, 0:256], ps[:, 0:256])
                        nc.scalar.copy(h_own[:, q * 256 : (q + 1) * 256], ps[:, 0:256])
                        nc.vector.tensor_copy(stgf[:, qi, 128:132], ps[:, 256:260])
                        nc.vector.tensor_copy(
                            alsrc_own[:, q * 4 : (q + 1) * 4], ps[:, 256:260]
                        )
                        nc.vector.tensor_copy(
                            aldst[:, q * 4 : (q + 1) * 4], ps[:, 260:264]
                        )
                    nc.sync.dma_start(
                        tsh3[:, q0 : q0 + nq, :], stg3[:, 0:nq, :]
                    )
                # self-loop weights, once per layer: exp(LRelu(alS_own+alD))
                nc.vector.tensor_add(esf_all[:], alsrc_own[:], aldst[:])
                nc.vector.tensor_scalar_min(esf_lr[:], esf_all[:], 0.0)
                nc.vector.tensor_scalar_max(esf_all[:], esf_all[:], 0.0)
                nc.vector.scalar_tensor_tensor(
                    esf_all[:],
                    esf_lr[:],
                    NEG,
                    esf_all[:],
                    op0=mybir.AluOpType.mult,
                    op1=mybir.AluOpType.add,
                )
                nc.scalar.activation(
                    esf_all[:], esf_all[:], mybir.ActivationFunctionType.Exp
                )
                if not full and estage < 9:
                    continue
                nc.gpsimd.collective_compute(
                    "AllGather",
                    mybir.AluOpType.bypass,
                    replica_groups=[list(range(NCORES))],
                    ins=[tableshard[0 : H1ROWS, :].opt()],
                    outs=[tabA.opt()],
                )
                # ---------------- edge phase -------------------------------
                if not full and estage < 10:
                    continue
                qctr = [0]

                # device limit: ≤1024 indices per dma_gather instruction
                def chunked_gather(hx, hx3, col0, ncols, tab, io):
                    # estage=14: timing probe — gather only the first
                    # 512B of each 768B row (h only, drops al_src)
                    probe = (not full) and estage == 14
                    ew = 256 if probe else ROWW
                    hv = (
                        hx[:, : (CMAXG * ROWW // 256) * 256].rearrange(
                            "p (c w) -> p c w", w=256
                        )
                        if probe
                        else hx3
                    )
                    tabv = tab[:, 0:256] if probe else tab
                    for k0 in range(0, ncols, 8):
                        kc = min(8, ncols - k0)
                        nc.gpsimd.dma_gather(
                            hv[:, col0 + k0 : col0 + k0 + kc, :],
                            tabv,
                            idx_sb[:, io + k0 * 8 : io + (k0 + kc) * 8],
                            kc * 128,
                            kc * 128,
                            ew,
                            elem_step=ROWW if probe else None,
                            queue_num=qctr[0] % 4,
                        )
                        qctr[0] += 1

                gstate = {}

                def issue_h1(gi):
                    t0, T, gLA, gLB = groups[gi]
                    ioA, ioB, mo = goffs[gi]
                    hx = gpool.tile([128, CMAXG * ROWW], BF, tag="hx")
                    hx3 = hx[:].rearrange("p (c w) -> p c w", w=ROWW)
                    gstate[gi] = (hx, hx3)
                    if gLA:
                        chunked_gather(hx, hx3, 0, T * gLA, tabA, ioA)

                def issue_h2(gi):
                    t0, T, gLA, gLB = groups[gi]
                    ioA, ioB, mo = goffs[gi]
                    hx, hx3 = gstate[gi]
                    if gLB:
                        chunked_gather(hx, hx3, T * gLA, T * gLB, tabB, ioB)

                def vector_ops(gi):
                    t0, T, gLA, gLB = groups[gi]
                    ioA, ioB, mo = goffs[gi]
                    CA, CB = T * gLA, T * gLB
                    C = CA + CB
                    hx, hx3 = gstate.pop(gi)

                    hxf = hx[:].bitcast(FP).rearrange("p (c w) -> p c w", w=ROWW // 2)
                    # alS[p, c, h] at f32 columns 128..132 of each row
                    e = epool.tile([128, CMAXG * 4], FP, tag="e")
                    if not full and estage in (10, 14):
                        nc.vector.tensor_copy(e[:, 0:64], hx3[:, 0, 0:64])
                        return
                    e3 = e[:].rearrange("p (c h) -> p c h", h=4)
                    ab = epool.tile([128, CMAXG * 4], BF, tag="ab")
                    ab3 = ab[:].rearrange("p (c h) -> p c h", h=4)

                    alD = aldst[:].rearrange("p (t h) -> p t h", h=4)[
                        :, t0 : t0 + T, :
                    ]

                    def reg_view(v3, off, L):
                        # [p, c, x] cols off..off+T*L -> [p, T, L, x]
                        return v3[:, off : off + T * L, :].rearrange(
                            "p (t j) h -> p t j h", j=L
                        )

                    regions = []
                    if gLA:
                        regions.append((0, gLA))
                    if gLB:
                        regions.append((CA, gLB))

                    # logits: e = al_src[src] + al_dst[dst]
                    for off, L in regions:
                        alS_r = hxf[:, off : off + T * L, 128:132].rearrange(
                            "p (t j) h -> p t j h", j=L
                        )
                        alD_b = alD.unsqueeze(2).broadcast_to((128, T, L, 4))
                        nc.vector.tensor_add(reg_view(e3, off, L), alS_r, alD_b)

                    eflat = e[:, : C * 4]
                    # leaky relu (composed: e = max(e,0) + NEG*min(e,0)), then pad mask
                    lr = epool.tile([128, CMAXG * 4], FP, name="lr", tag="lr")
                    lrf = lr[:, : C * 4]
                    nc.vector.tensor_scalar_min(lrf, eflat, 0.0)
                    nc.vector.tensor_scalar_max(eflat, eflat, 0.0)
                    nc.vector.scalar_tensor_tensor(
                        eflat,
                        lrf,
                        NEG,
                        eflat,
                        op0=mybir.AluOpType.mult,
                        op1=mybir.AluOpType.add,
                    )
                    mask_b = (
                        mask_sb[:, mo : mo + C].unsqueeze(2).broadcast_to((128, C, 4))
                    )
                    nc.vector.tensor_add(e3[:, 0:C, :], e3[:, 0:C, :], mask_b)

                    # no max-shift: logits are small (|e| < 8 for these
                    # inputs), so exp(e) is safe in f32 with huge margin and
                    # the whole segment-max pass is skipped
                    nc.scalar.activation(
                        eflat, eflat, mybir.ActivationFunctionType.Exp
                    )
                    esfv = esf_all[:, t0 * 4 : (t0 + T) * 4]

                    # denom and reciprocal
                    dt_ = []
                    for off, L in regions:
                        d_r = epool.tile([128, TMAXG * 4], FP, name=f"d{off == 0}", tag=f"d{off == 0}")
                        in_r = (
                            e3[:, off : off + T * L, :]
                            .rearrange("p (t j) h -> p t h j", j=L)
                        )
                        nc.vector.reduce_sum(
                            d_r[:, : T * 4], in_r, axis=mybir.AxisListType.X
                        )
                        dt_.append(d_r)
                    den = dt_[0]
                    if len(dt_) == 2:
                        nc.vector.tensor_add(
                            den[:, : T * 4], den[:, : T * 4], dt_[1][:, : T * 4]
                        )
                    nc.vector.tensor_add(den[:, : T * 4], den[:, : T * 4], esfv)
                    rec = epool.tile([128, TMAXG * 4], FP, tag="rec")
                    nc.vector.reciprocal(rec[:, : T * 4], den[:, : T * 4])
                    r3 = rec[:].rearrange("p (t h) -> p t h", h=4)[:, 0:T, :]
                    # alpha_self = ex_self / denom
                    nc.vector.tensor_mul(esfv, esfv, rec[:, : T * 4])

                    # alpha = ex / denom, cast to bf16
                    for off, L in regions:
                        r_b = r3.unsqueeze(2).broadcast_to((128, T, L, 4))
                        nc.vector.tensor_mul(
                            reg_view(e3, off, L), reg_view(e3, off, L), r_b
                        )
                    nc.vector.tensor_copy(ab[:, : C * 4], eflat)

                    if not full and estage == 11:
                        return
                    # messages: hx[:, :, 0:256] *= alpha (broadcast over 64)
                    h4 = hx3[:, 0:C, 0:256].rearrange("p c (h d) -> p c h d", d=D)
                    a4 = ab3[:, 0:C, :].unsqueeze(3).broadcast_to((128, C, 4, D))
                    nc.vector.tensor_mul(h4, h4, a4)
                    if not full and estage == 15:
                        return

                    # segment sum -> [p, T, 256]. Pre-halve with contiguous
                    # in-place adds (fast DVE mode) so the strided reduce only
                    # sees ~L/4 columns.
                    og = ogpool.tile([128, TMAXG * 256], FP, tag="ogA")
                    ogt = []
                    for off, L in regions:
                        o_r = (
                            og
                            if not ogt
                            else ogpool.tile(
                                [128, TMAXG * 256], FP, name="ogB", tag="ogB"
                            )
                        )
                        v = hx3[:, off : off + T * L, 0:256].rearrange(
                            "p (t j) f -> p t j f", j=L
                        )
                        L2 = L
                        for _ in range(2):
                            if L2 < 4:
                                break
                            h2 = L2 // 2
                            if L2 % 2:
                                nc.vector.tensor_add(
                                    v[:, :, 0, :], v[:, :, 0, :], v[:, :, L2 - 1, :]
                                )
                            nc.vector.tensor_add(
                                v[:, :, 0:h2, :],
                                v[:, :, 0:h2, :],
                                v[:, :, h2 : 2 * h2, :],
                            )
                            L2 = h2
                        in_r = v[:, :, 0:L2, :].rearrange("p t j f -> p t f j")
                        nc.vector.reduce_sum(
                            o_r[:, : T * 256], in_r, axis=mybir.AxisListType.X
                        )
                        ogt.append(o_r)
                    if len(ogt) == 2:
                        nc.vector.tensor_add(
                            og[:, : T * 256], og[:, : T * 256], ogt[1][:, : T * 256]
                        )
                    if not full and estage == 16:
                        return
                    # self message: og += h_own * alpha_self (relu_f doubles
                    # as the scratch; it is rewritten by the relu below)
                    relu_f = ogpool.tile([128, TMAXG * 256], FP, tag="reluf")
                    ho4 = h_own[:, t0 * 256 : (t0 + T) * 256].rearrange(
                        "p (t h d) -> p t h d", h=4, d=D
                    )
                    as4 = (
                        esf_all[:]
                        .rearrange("p (t h) -> p t h", h=4)[:, t0 : t0 + T, :]
                        .unsqueeze(3)
                        .broadcast_to((128, T, 4, D))
                    )
                    sm4 = relu_f[:].rearrange("p (t h d) -> p t h d", h=4, d=D)[
                        :, 0:T
                    ]
                    nc.vector.tensor_mul(sm4, ho4, as4)
                    nc.vector.tensor_add(
                        og[:, : T * 256], og[:, : T * 256], relu_f[:, : T * 256]
                    )

                    # bias + relu
                    og3 = og[:].rearrange("p (t f) -> p t f", f=256)
                    bias_b = bias_sb[l][:].unsqueeze(1).broadcast_to((128, T, 256))
                    nc.vector.tensor_add(og3[:, 0:T, :], og3[:, 0:T, :], bias_b)
                    nc.scalar.activation(
                        relu_f[:, : T * 256],
                        og[:, : T * 256],
                        mybir.ActivationFunctionType.Relu,
                    )

                    if not full and estage == 12:
                        return
                    relu_b = ogpool.tile([128, TMAXG * 256], BF, tag="relub")
                    nc.vector.tensor_copy(
                        relu_b[:, : T * 256], relu_f[:, : T * 256]
                    )
                    rb3 = relu_b[:].rearrange("p (t f) -> p t f", f=256)
                    if l < 2:
                        for ti in range(T):
                            for fb, xt in ((0, xT_a), (1, xT_b)):
                                pt = pstp.tile([128, 128], BF, tag="psT")
                                nc.tensor.transpose(
                                    pt[:],
                                    rb3[:, ti, fb * 128 : (fb + 1) * 128],
                                    ident[:],
                                )
                                nc.scalar.copy(
                                    xt[:, (t0 + ti) * 128 : (t0 + ti + 1) * 128],
                                    pt[:],
                                )
                    elif full:
                        for ti in range(T):
                            q = t0 + ti
                            nc.tensor.matmul(
                                pool_ps[:],
                                onehot_sb[:, q * G : (q + 1) * G],
                                rb3[:, ti, :],
                                start=(q == 0),
                                stop=(q == TILES - 1),
                            )

                # Software pipeline: keep H1 gathers KPRE groups ahead so the
                # Pool-engine stream never stalls at the H2-allgather wait
                # (an in-order stall there would idle every DMA queue).
                KPRE = 2
                NG = len(groups)
                for g in range(min(KPRE, NG)):
                    issue_h1(g)
                nc.gpsimd.collective_compute(
                    "AllGather",
                    mybir.AluOpType.bypass,
                    replica_groups=[list(range(NCORES))],
                    ins=[tableshard[H1ROWS:SHARD, :].opt()],
                    outs=[tabB.opt()],
                )
                for g in range(NG):
                    issue_h2(g)
                    if g + KPRE < NG:
                        issue_h1(g + KPRE)
                    vector_ops(g)

            pout = cpool.tile([64, 256], FP, tag="pout")
            if full:
                nc.vector.tensor_copy(pout[:], pool_ps[:])
            else:
                nc.vector.memset(pout[:], 0.0)
                nc.vector.tensor_add(pout[:, 0:196], pout[:, 0:196], aldst[0:64, 0:196])
            nc.sync.dma_start(d_out[:], pout[:])

    nc.compile()
    return nc


# ----------------------------------------------------------------------------
# Entry point
# ----------------------------------------------------------------------------

def _prepare(inputs):
    key = (
        inputs["edge_index"].tobytes(),
        inputs["batch"].tobytes(),
    )
    kh = hash(key)
    if kh in _cache:
        return _cache[kh]
    edge_index = np.asarray(inputs["edge_index"], np.int64)
    batch = np.asarray(inputs["batch"], np.int64)
    meta = _preprocess(edge_index, batch)
    nc = _build_program(meta)
    _cache[kh] = (meta, nc)
    return meta, nc


def _make_inmaps(inputs, meta):
    x = np.asarray(inputs["x"], np.float32)
    batch = np.asarray(inputs["batch"], np.int64)
    core_nodes = meta["core_nodes"]

    wcats = []
    biases = []
    for l in range(3):
        Wl = np.asarray(inputs[f"W{l}"], np.float64)
        wcats.append(
            _build_wcat(
                Wl,
                np.asarray(inputs[f"a_src{l}"], np.float64),
                np.asarray(inputs[f"a_dst{l}"], np.float64),
            )
        )
        b = np.asarray(inputs[f"b{l}"], np.float32)
        biases.append(np.tile(b[None, :], (128, 1)).astype(np.float32))
    ident = np.eye(128, dtype=BFNP)

    in_maps = []
    for c in range(NCORES):
        nodes = core_nodes[c]
        safe = np.maximum(nodes, 0)
        x0 = x[safe]
        x0[nodes < 0] = 0.0
        # column q*128+p = node (tile q, partition p); core_nodes is tile-major
        x0T = np.ascontiguousarray(x0.T).astype(BFNP)
        in_maps.append(
            {
                "x0T": x0T,
                "wcat0": wcats[0],
                "wcat1": wcats[1],
                "wcat2": wcats[2],
                "bias0": biases[0],
                "bias1": biases[1],
                "bias2": biases[2],
                "ident": ident,
                "idxall": meta["idx_all"][c],
                "maskall": meta["mask_all"][c],
                "onehot": meta["onehot"][c].astype(BFNP),
            }
        )
    return in_maps


def _run(inputs, trace=False):
    meta, nc = _prepare(inputs)
    in_maps = _make_inmaps(inputs, meta)
    res = run_bass_kernel_spmd(
        nc, in_maps, core_ids=list(range(NCORES)), trace=trace
    )
    out = np.zeros((G, HD), np.float64)
    for c in range(NCORES):
        out += res.results[c]["pooled"].astype(np.float64)
    return out.astype(np.float32), res


def kernel(**inputs) -> np.ndarray:
    out, _ = _run(inputs, trace=False)
    return out


def kernel_traced(**inputs):
    out, res = _run(inputs, trace=True)
    return out, res

